# revision 17
# baseline (speedup 1.0000x reference)
"""Trainium2 Bass kernel for nn_DecodeSBP (keypoint heatmap decode).

Contract: kernel(x=[1,133,512,512] f32) -> [133,3] f32
  joints[k] = (4*xx, 4*yy, conf) if conf > 0.8 else (-4, -4, -1)
  where flat = argmax(sigmoid(x[0,k])), conf = sigmoid(max), yy = flat//512,
  xx = flat%512. sigmoid is monotonic so the argmax runs on raw logits.

Sharding: keypoint dim across 8 cores (17/core, core 7 zero-padded).
Per-core program: stream 17 MB through SBUF with one DVE reduce_max pass
(per-partition max per keypoint) -> TensorE transpose -> global max ->
mask*reversed-iota argmax idiom for winning partition -> row gather via
register-offset DMAs -> same idiom for the index within the row -> decode.
The finale runs in two halves so the first half hides under streaming.
"""

import sys
from contextlib import ExitStack

for _p in ("/opt/trn_rl_repo", "/opt/pypackages"):
    if _p not in sys.path:
        sys.path.append(_p)

import numpy as np

import concourse.bacc as bacc
import concourse.bass as bass
import concourse.tile as tile
from concourse import mybir
from concourse.bass_utils import run_bass_kernel_spmd
from concourse.masks import make_identity

K = 17          # keypoints per core
NK = 133        # total keypoints
ROW = 262144    # 512*512
P = 128         # SBUF partitions
F = ROW // P    # 2048 free elems per partition
W = 512
N_CORES = 8
KSPLIT = 9      # finale half 1 covers keypoints [0, KSPLIT)

f32 = mybir.dt.float32
i32 = mybir.dt.int32
Alu = mybir.AluOpType
Act = mybir.ActivationFunctionType

_NC_CACHE = None


def _build(kp_per_tile=3, bufs=6):
    nc = bacc.Bacc("TRN2", target_bir_lowering=False, debug=False)
    x_dram = nc.dram_tensor("x", [K, ROW], f32, kind="ExternalInput")
    out_dram = nc.dram_tensor("out", [K, 3], f32, kind="ExternalOutput")

    x_pkf = x_dram.ap().rearrange("k (p f) -> p k f", f=F)      # [128, K, 2048]
    x_flat = x_dram.ap().rearrange("k f -> (k f)")

    with tile.TileContext(nc) as tc, ExitStack() as ctx:
        const_pool = ctx.enter_context(tc.tile_pool(name="const", bufs=1))
        in_pool = ctx.enter_context(tc.tile_pool(name="in", bufs=bufs))
        small_pool = ctx.enter_context(tc.tile_pool(name="small", bufs=1))
        psum_pool = ctx.enter_context(
            tc.tile_pool(name="psum", bufs=1, space="PSUM"))

        ident = const_pool.tile([P, P], f32)
        make_identity(nc, ident[:])
        # riota_p[k, j] = 128 - j
        riota_p = const_pool.tile([K, P], f32)
        nc.gpsimd.iota(riota_p[:], pattern=[[-1, P]], base=P,
                       channel_multiplier=0,
                       allow_small_or_imprecise_dtypes=True)
        # riota_j[k, j] = 2048 - j
        riota_j = const_pool.tile([K, F], f32)
        nc.gpsimd.iota(riota_j[:], pattern=[[-1, F]], base=F,
                       channel_multiplier=0,
                       allow_small_or_imprecise_dtypes=True)

        pmax = small_pool.tile([P, K], f32)

        def stream(k_lo, k_hi):
            """DMA + per-partition reduce for keypoints [k_lo, k_hi)."""
            k0 = k_lo
            i = 0
            while k0 < k_hi:
                g = min(kp_per_tile, k_hi - k0)
                t = in_pool.tile([P, g * F], f32, tag="xin")
                eng = nc.sync if (k0 // kp_per_tile) % 2 == 0 else nc.scalar
                eng.dma_start(
                    t[:].rearrange("p (g f) -> p g f", f=F),
                    x_pkf[:, k0:k0 + g, :])
                nc.vector.reduce_max(
                    pmax[:, k0:k0 + g],
                    t[:].rearrange("p (g f) -> p g f", f=F),
                    axis=mybir.AxisListType.X)
                k0 += g
                i += 1

        def finale(h, k_lo, k_hi, engines):
            """argmax + gather + decode for keypoints [k_lo, k_hi).
            Writes out rows [k_lo, k_hi) of the output."""
            kh = k_hi - k_lo

            # transpose this half's pmax columns -> [kh, 128]
            psumT = psum_pool.tile([kh, P], f32, tag=f"psumT{h}")
            nc.tensor.matmul(psumT[:], pmax[:, k_lo:k_hi], ident[:],
                             is_transpose=True)
            pT = small_pool.tile([kh, P], f32, tag=f"pT{h}")
            nc.vector.tensor_copy(pT[:], psumT[:])

            gmax = small_pool.tile([kh, 1], f32, tag=f"gmax{h}")
            nc.vector.reduce_max(gmax[:], pT[:], axis=mybir.AxisListType.X)
            cand_p = small_pool.tile([kh, P], f32, tag=f"cand_p{h}")
            nc.vector.scalar_tensor_tensor(
                cand_p[:], in0=pT[:], scalar=gmax[:], in1=riota_p[0:kh, :],
                op0=Alu.is_ge, op1=Alu.mult)
            rp = small_pool.tile([kh, 1], f32, tag=f"rp{h}")  # 128 - p*
            nc.vector.reduce_max(rp[:], cand_p[:], axis=mybir.AxisListType.X)

            # DRAM element offset of winning row k: (128*(k+1) - rp)*2048
            psumR = psum_pool.tile([1, kh], f32, tag=f"psumR{h}")
            nc.tensor.matmul(psumR[:], rp[:], ident[0:kh, 0:kh],
                             is_transpose=True)
            kiota_row = const_pool.tile([1, kh], f32, tag=f"kiota{h}")
            nc.gpsimd.iota(kiota_row[:], pattern=[[P, kh]], base=P * (k_lo + 1),
                           channel_multiplier=0,
                           allow_small_or_imprecise_dtypes=True)
            offs_f = small_pool.tile([1, kh], f32, tag=f"offs_f{h}")
            nc.vector.tensor_sub(offs_f[:], kiota_row[:], psumR[:])
            nc.vector.tensor_scalar(offs_f[:], offs_f[:], float(F), None,
                                    Alu.mult)
            offs_i = small_pool.tile([1, kh], i32, tag=f"offs_i{h}")
            nc.vector.tensor_copy(offs_i[:], offs_f[:])

            grow = small_pool.tile([kh, F], f32, tag=f"grow{h}")
            dsem = nc.alloc_semaphore(f"gather_dma{h}")
            with tc.tile_critical():
                for k in range(kh):
                    eng = engines[k % len(engines)]
                    reg = eng.alloc_register()
                    eng.load(reg, offs_i[0:1, k:k + 1])
                    off = eng.snap(reg, donate=True)
                    eng.dma_start(
                        grow[k:k + 1, :],
                        x_flat[bass.ds(off, F)]).then_inc(dsem, 16)
                engines[0].wait_ge(dsem, kh * 16)

            # index within the winning row
            cand_j = small_pool.tile([kh, F], f32, tag=f"cand_j{h}")
            nc.vector.scalar_tensor_tensor(
                cand_j[:], in0=grow[:], scalar=gmax[:], in1=riota_j[0:kh, :],
                op0=Alu.is_ge, op1=Alu.mult)
            rj = small_pool.tile([kh, 1], f32, tag=f"rj{h}")  # 2048 - j*
            nc.vector.reduce_max(rj[:], cand_j[:], axis=mybir.AxisListType.X)

            # decode: flat = p* * 2048 + j* = 264192 - 2048*rp - rj
            flat = small_pool.tile([kh, 1], f32, tag=f"flat{h}")
            nc.vector.scalar_tensor_tensor(
                flat[:], in0=rp[:], scalar=float(F), in1=rj[:],
                op0=Alu.mult, op1=Alu.add)
            nc.vector.tensor_scalar(flat[:], flat[:], -1.0, float(P * F + F),
                                    Alu.mult, Alu.add)
            flat_i = small_pool.tile([kh, 1], i32, tag=f"flat_i{h}")
            nc.vector.tensor_copy(flat_i[:], flat[:])
            xx_i = small_pool.tile([kh, 1], i32, tag=f"xx_i{h}")
            nc.vector.tensor_scalar(xx_i[:], flat_i[:], W - 1, None,
                                    Alu.bitwise_and)
            yy_i = small_pool.tile([kh, 1], i32, tag=f"yy_i{h}")
            nc.vector.tensor_scalar(yy_i[:], flat_i[:], 9, None,
                                    Alu.logical_shift_right)

            conf = small_pool.tile([kh, 1], f32, tag=f"conf{h}")
            nc.scalar.activation(conf[:], gmax[:], Act.Sigmoid)
            valid = small_pool.tile([kh, 1], f32, tag=f"valid{h}")
            nc.vector.tensor_scalar(valid[:], conf[:], 0.8, None, Alu.is_gt)

            # candidate values [4*xx, 4*yy, conf]; defaults (-4,-4,-1)
            cand = small_pool.tile([kh, 3], f32, tag=f"cand{h}")
            nc.vector.tensor_scalar(cand[:, 0:1], xx_i[:], 4.0, None, Alu.mult)
            nc.vector.tensor_scalar(cand[:, 1:2], yy_i[:], 4.0, None, Alu.mult)
            nc.vector.tensor_copy(cand[:, 2:3], conf[:])
            vb3 = small_pool.tile([kh, 3], i32, tag=f"vb3{h}")
            nc.vector.tensor_scalar(vb3[:], cand[:], 0.0, valid[:],
                                    Alu.mult, Alu.add)
            out_sb = small_pool.tile([kh, 3], f32, tag=f"out_sb{h}")
            nc.vector.memset(out_sb[:, 0:2], -4.0)
            nc.vector.memset(out_sb[:, 2:3], -1.0)
            nc.vector.copy_predicated(out_sb[:], vb3[:], cand[:])
            nc.sync.dma_start(out_dram.ap()[k_lo:k_hi, :], out_sb[:])

        stream(0, KSPLIT)
        # half 1 finale: gather on gpsimd only (SWDGE) so its DMAs don't
        # queue behind half-2 streaming on the sync/scalar HWDGE rings.
        finale(0, 0, KSPLIT, [nc.gpsimd])
        stream(KSPLIT, K)
        finale(1, KSPLIT, K, [nc.sync, nc.scalar])

    nc.compile()
    return nc


def _get_nc():
    global _NC_CACHE
    if _NC_CACHE is None:
        _NC_CACHE = _build()
    return _NC_CACHE


def _shard(x: np.ndarray) -> list[dict[str, np.ndarray]]:
    xf = np.ascontiguousarray(np.asarray(x, dtype=np.float32).reshape(NK, ROW))
    shards = []
    for c in range(N_CORES):
        lo = c * K
        s = xf[lo:min(lo + K, NK)]
        if s.shape[0] < K:
            s = np.concatenate(
                [s, np.zeros((K - s.shape[0], ROW), np.float32)], axis=0)
        shards.append({"x": np.ascontiguousarray(s)})
    return shards


def _run(x, trace=False, **kw):
    nc = _get_nc()
    res = run_bass_kernel_spmd(nc, _shard(x), core_ids=list(range(N_CORES)),
                               trace=trace, **kw)
    out = np.concatenate([r["out"] for r in res.results], axis=0)[:NK]
    return out.astype(np.float32), res


def kernel(x: np.ndarray) -> np.ndarray:
    out, _ = _run(x, trace=False)
    return out


# revision 18
# speedup vs baseline: 1.1120x; 1.1120x over previous
"""Trainium2 Bass kernel for nn_DecodeSBP (keypoint heatmap decode).

Contract: kernel(x=[1,133,512,512] f32) -> [133,3] f32
  joints[k] = (4*xx, 4*yy, conf) if conf > 0.8 else (-4, -4, -1)
  where flat = argmax(sigmoid(x[0,k])), conf = sigmoid(max), yy = flat//512,
  xx = flat%512. sigmoid is monotonic so the argmax runs on raw logits.

Sharding: keypoint dim across 8 cores (17/core, core 7 zero-padded).
Per-core program: stream 17 MB through SBUF with one DVE reduce_max pass
(per-partition max per keypoint) -> TensorE transpose -> global max ->
mask*reversed-iota argmax idiom for winning partition -> row gather via
register-offset DMAs -> same idiom for the index within the row -> decode.
The finale runs in two halves so the first half hides under streaming.
"""

import sys
from contextlib import ExitStack

for _p in ("/opt/trn_rl_repo", "/opt/pypackages"):
    if _p not in sys.path:
        sys.path.append(_p)

import numpy as np

import concourse.bacc as bacc
import concourse.bass as bass
import concourse.tile as tile
from concourse import mybir
from concourse.bass_utils import run_bass_kernel_spmd
from concourse.masks import make_identity

K = 17          # keypoints per core
NK = 133        # total keypoints
ROW = 262144    # 512*512
P = 128         # SBUF partitions
F = ROW // P    # 2048 free elems per partition
W = 512
N_CORES = 8
KSPLIT = 9      # finale half 1 covers keypoints [0, KSPLIT)

f32 = mybir.dt.float32
i32 = mybir.dt.int32
Alu = mybir.AluOpType
Act = mybir.ActivationFunctionType

_NC_CACHE = None


def _build(kp_per_tile=3, bufs=6):
    nc = bacc.Bacc("TRN2", target_bir_lowering=False, debug=False)
    x_dram = nc.dram_tensor("x", [K, ROW], f32, kind="ExternalInput")
    out_dram = nc.dram_tensor("out", [K, 3], f32, kind="ExternalOutput")

    x_pkf = x_dram.ap().rearrange("k (p f) -> p k f", f=F)      # [128, K, 2048]
    x_flat = x_dram.ap().rearrange("k f -> (k f)")

    with tile.TileContext(nc) as tc, ExitStack() as ctx:
        const_pool = ctx.enter_context(tc.tile_pool(name="const", bufs=1))
        in_pool = ctx.enter_context(tc.tile_pool(name="in", bufs=bufs))
        small_pool = ctx.enter_context(tc.tile_pool(name="small", bufs=1))
        psum_pool = ctx.enter_context(
            tc.tile_pool(name="psum", bufs=1, space="PSUM"))

        ident = const_pool.tile([P, P], f32)
        make_identity(nc, ident[:])
        # riota_p[k, j] = 128 - j
        riota_p = const_pool.tile([K, P], f32)
        nc.gpsimd.iota(riota_p[:], pattern=[[-1, P]], base=P,
                       channel_multiplier=0,
                       allow_small_or_imprecise_dtypes=True)
        # riota_j[k, j] = 2048 - j
        riota_j = const_pool.tile([K, F], f32)
        nc.gpsimd.iota(riota_j[:], pattern=[[-1, F]], base=F,
                       channel_multiplier=0,
                       allow_small_or_imprecise_dtypes=True)

        pmax_a = small_pool.tile([P, KSPLIT], f32)
        pmax_b = small_pool.tile([P, K - KSPLIT], f32)

        def stream(k_lo, k_hi):
            """DMA + per-partition reduce for keypoints [k_lo, k_hi)."""
            k0 = k_lo
            i = 0
            while k0 < k_hi:
                g = min(kp_per_tile, k_hi - k0)
                t = in_pool.tile([P, g * F], f32, tag="xin")
                nc.sync.dma_start(
                    t[:].rearrange("p (g f) -> p g f", f=F),
                    x_pkf[:, k0:k0 + g, :])
                pm, ofs = (pmax_a, 0) if k_lo < KSPLIT else (pmax_b, KSPLIT)
                nc.vector.reduce_max(
                    pm[:, k0 - ofs:k0 - ofs + g],
                    t[:].rearrange("p (g f) -> p g f", f=F),
                    axis=mybir.AxisListType.X)
                k0 += g
                i += 1

        def finale(h, k_lo, k_hi, engines):
            """argmax + gather + decode for keypoints [k_lo, k_hi).
            Writes out rows [k_lo, k_hi) of the output."""
            kh = k_hi - k_lo

            # transpose this half's pmax -> [kh, 128]
            pm = pmax_a if h == 0 else pmax_b
            psumT = psum_pool.tile([kh, P], f32, tag=f"psumT{h}")
            nc.tensor.matmul(psumT[:], pm[:], ident[:],
                             is_transpose=True)
            pT = small_pool.tile([kh, P], f32, tag=f"pT{h}")
            nc.vector.tensor_copy(pT[:], psumT[:])

            gmax = small_pool.tile([kh, 1], f32, tag=f"gmax{h}")
            nc.vector.reduce_max(gmax[:], pT[:], axis=mybir.AxisListType.X)
            cand_p = small_pool.tile([kh, P], f32, tag=f"cand_p{h}")
            nc.vector.scalar_tensor_tensor(
                cand_p[:], in0=pT[:], scalar=gmax[:], in1=riota_p[0:kh, :],
                op0=Alu.is_ge, op1=Alu.mult)
            rp = small_pool.tile([kh, 1], f32, tag=f"rp{h}")  # 128 - p*
            nc.vector.reduce_max(rp[:], cand_p[:], axis=mybir.AxisListType.X)

            # DRAM element offset of winning row k: (128*(k+1) - rp)*2048
            psumR = psum_pool.tile([1, kh], f32, tag=f"psumR{h}")
            nc.tensor.matmul(psumR[:], rp[:], ident[0:kh, 0:kh],
                             is_transpose=True)
            kiota_row = const_pool.tile([1, kh], f32, tag=f"kiota{h}")
            nc.gpsimd.iota(kiota_row[:], pattern=[[P, kh]], base=P * (k_lo + 1),
                           channel_multiplier=0,
                           allow_small_or_imprecise_dtypes=True)
            offs_f = small_pool.tile([1, kh], f32, tag=f"offs_f{h}")
            nc.vector.tensor_sub(offs_f[:], kiota_row[:], psumR[:])
            nc.vector.tensor_scalar(offs_f[:], offs_f[:], float(F), None,
                                    Alu.mult)
            offs_i = small_pool.tile([1, kh], i32, tag=f"offs_i{h}")
            nc.vector.tensor_copy(offs_i[:], offs_f[:])

            grow = small_pool.tile([kh, F], f32, tag=f"grow{h}")
            dsem = nc.alloc_semaphore(f"gather_dma{h}")
            with tc.tile_critical():
                for k in range(kh):
                    eng = engines[k % len(engines)]
                    reg = eng.alloc_register()
                    eng.load(reg, offs_i[0:1, k:k + 1])
                    off = eng.snap(reg, donate=True)
                    eng.dma_start(
                        grow[k:k + 1, :],
                        x_flat[bass.ds(off, F)]).then_inc(dsem, 16)
                engines[0].wait_ge(dsem, kh * 16)

            # index within the winning row
            cand_j = small_pool.tile([kh, F], f32, tag=f"cand_j{h}")
            nc.vector.scalar_tensor_tensor(
                cand_j[:], in0=grow[:], scalar=gmax[:], in1=riota_j[0:kh, :],
                op0=Alu.is_ge, op1=Alu.mult)
            rj = small_pool.tile([kh, 1], f32, tag=f"rj{h}")  # 2048 - j*
            nc.vector.reduce_max(rj[:], cand_j[:], axis=mybir.AxisListType.X)

            # decode: flat = p* * 2048 + j* = 264192 - 2048*rp - rj
            flat = small_pool.tile([kh, 1], f32, tag=f"flat{h}")
            nc.vector.scalar_tensor_tensor(
                flat[:], in0=rp[:], scalar=float(F), in1=rj[:],
                op0=Alu.mult, op1=Alu.add)
            nc.vector.tensor_scalar(flat[:], flat[:], -1.0, float(P * F + F),
                                    Alu.mult, Alu.add)
            flat_i = small_pool.tile([kh, 1], i32, tag=f"flat_i{h}")
            nc.vector.tensor_copy(flat_i[:], flat[:])
            xx_i = small_pool.tile([kh, 1], i32, tag=f"xx_i{h}")
            nc.vector.tensor_scalar(xx_i[:], flat_i[:], W - 1, None,
                                    Alu.bitwise_and)
            yy_i = small_pool.tile([kh, 1], i32, tag=f"yy_i{h}")
            nc.vector.tensor_scalar(yy_i[:], flat_i[:], 9, None,
                                    Alu.logical_shift_right)

            conf = small_pool.tile([kh, 1], f32, tag=f"conf{h}")
            nc.scalar.activation(conf[:], gmax[:], Act.Sigmoid)
            valid = small_pool.tile([kh, 1], f32, tag=f"valid{h}")
            nc.vector.tensor_scalar(valid[:], conf[:], 0.8, None, Alu.is_gt)

            # candidate values [4*xx, 4*yy, conf]; defaults (-4,-4,-1)
            cand = small_pool.tile([kh, 3], f32, tag=f"cand{h}")
            nc.vector.tensor_scalar(cand[:, 0:1], xx_i[:], 4.0, None, Alu.mult)
            nc.vector.tensor_scalar(cand[:, 1:2], yy_i[:], 4.0, None, Alu.mult)
            nc.vector.tensor_copy(cand[:, 2:3], conf[:])
            vb3 = small_pool.tile([kh, 3], i32, tag=f"vb3{h}")
            nc.vector.tensor_scalar(vb3[:], cand[:], 0.0, valid[:],
                                    Alu.mult, Alu.add)
            out_sb = small_pool.tile([kh, 3], f32, tag=f"out_sb{h}")
            nc.vector.memset(out_sb[:, 0:2], -4.0)
            nc.vector.memset(out_sb[:, 2:3], -1.0)
            nc.vector.copy_predicated(out_sb[:], vb3[:], cand[:])
            nc.scalar.dma_start(out_dram.ap()[k_lo:k_hi, :], out_sb[:])

        stream(0, KSPLIT)
        # half 1 finale: gather on gpsimd only (SWDGE) so its DMAs don't
        # queue behind half-2 streaming on the sync/scalar HWDGE rings.
        finale(0, 0, KSPLIT, [nc.gpsimd])
        stream(KSPLIT, K)
        finale(1, KSPLIT, K, [nc.sync, nc.scalar])

    nc.compile()
    return nc


def _get_nc():
    global _NC_CACHE
    if _NC_CACHE is None:
        _NC_CACHE = _build()
    return _NC_CACHE


def _shard(x: np.ndarray) -> list[dict[str, np.ndarray]]:
    xf = np.ascontiguousarray(np.asarray(x, dtype=np.float32).reshape(NK, ROW))
    shards = []
    for c in range(N_CORES):
        lo = c * K
        s = xf[lo:min(lo + K, NK)]
        if s.shape[0] < K:
            s = np.concatenate(
                [s, np.zeros((K - s.shape[0], ROW), np.float32)], axis=0)
        shards.append({"x": np.ascontiguousarray(s)})
    return shards


def _run(x, trace=False, **kw):
    nc = _get_nc()
    res = run_bass_kernel_spmd(nc, _shard(x), core_ids=list(range(N_CORES)),
                               trace=trace, **kw)
    out = np.concatenate([r["out"] for r in res.results], axis=0)[:NK]
    return out.astype(np.float32), res


def kernel(x: np.ndarray) -> np.ndarray:
    out, _ = _run(x, trace=False)
    return out


# revision 19
# speedup vs baseline: 1.4118x; 1.2696x over previous
"""Trainium2 Bass kernel for nn_DecodeSBP (keypoint heatmap decode).

Contract: kernel(x=[1,133,512,512] f32) -> [133,3] f32
  joints[k] = (4*xx, 4*yy, conf) if conf > 0.8 else (-4, -4, -1)
  where flat = argmax(sigmoid(x[0,k])), conf = sigmoid(max), yy = flat//512,
  xx = flat%512. sigmoid is monotonic so the argmax runs on raw logits.

Sharding: keypoint dim across 8 cores (17/core, core 7 zero-padded).
Per-core program: stream 17 MB through SBUF with one DVE reduce_max pass
(per-partition max per keypoint) -> TensorE transpose -> global max ->
mask*reversed-iota argmax idiom for winning partition -> row gather via
register-offset DMAs -> same idiom for the index within the row -> decode.
The finale runs in two halves so the first half hides under streaming.
"""

import sys
from contextlib import ExitStack

for _p in ("/opt/trn_rl_repo", "/opt/pypackages"):
    if _p not in sys.path:
        sys.path.append(_p)

import numpy as np

import concourse.bacc as bacc
import concourse.bass as bass
import concourse.tile as tile
from concourse import mybir
from concourse.bass_utils import run_bass_kernel_spmd
from concourse.masks import make_identity

K = 17          # keypoints per core
NK = 133        # total keypoints
ROW = 262144    # 512*512
P = 128         # SBUF partitions
F = ROW // P    # 2048 free elems per partition
W = 512
N_CORES = 8
KSPLIT = 9      # finale half 1 covers keypoints [0, KSPLIT)

f32 = mybir.dt.float32
i32 = mybir.dt.int32
Alu = mybir.AluOpType
Act = mybir.ActivationFunctionType

_NC_CACHE = None


def _build(kp_per_tile=3, bufs=6):
    nc = bacc.Bacc("TRN2", target_bir_lowering=False, debug=False)
    x_dram = nc.dram_tensor("x", [K, ROW], f32, kind="ExternalInput")
    out_dram = nc.dram_tensor("out", [K, 3], f32, kind="ExternalOutput")

    x_pkf = x_dram.ap().rearrange("k (p f) -> p k f", f=F)      # [128, K, 2048]
    x_flat = x_dram.ap().rearrange("k f -> (k f)")

    with tile.TileContext(nc) as tc, ExitStack() as ctx:
        const_pool = ctx.enter_context(tc.tile_pool(name="const", bufs=1))
        in_pool = ctx.enter_context(tc.tile_pool(name="in", bufs=bufs))
        small_pool = ctx.enter_context(tc.tile_pool(name="small", bufs=1))
        psum_pool = ctx.enter_context(
            tc.tile_pool(name="psum", bufs=1, space="PSUM"))

        ident = const_pool.tile([P, P], f32)
        make_identity(nc, ident[:])
        # riota_p[k, j] = 128 - j
        riota_p = const_pool.tile([K, P], f32)
        nc.gpsimd.iota(riota_p[:], pattern=[[-1, P]], base=P,
                       channel_multiplier=0,
                       allow_small_or_imprecise_dtypes=True)
        # riota_j[k, j] = 2048 - j
        riota_j = const_pool.tile([K, F], f32)
        nc.gpsimd.iota(riota_j[:], pattern=[[-1, F]], base=F,
                       channel_multiplier=0,
                       allow_small_or_imprecise_dtypes=True)

        pmax_a = small_pool.tile([P, KSPLIT], f32)
        pmax_b = small_pool.tile([P, K - KSPLIT], f32)

        def stream(k_lo, k_hi):
            """DMA + per-partition reduce for keypoints [k_lo, k_hi)."""
            k0 = k_lo
            i = 0
            while k0 < k_hi:
                g = min(kp_per_tile, k_hi - k0)
                t = in_pool.tile([P, g * F], f32, tag="xin")
                nc.sync.dma_start(
                    t[:].rearrange("p (g f) -> p g f", f=F),
                    x_pkf[:, k0:k0 + g, :])
                pm, ofs = (pmax_a, 0) if k_lo < KSPLIT else (pmax_b, KSPLIT)
                nc.vector.reduce_max(
                    pm[:, k0 - ofs:k0 - ofs + g],
                    t[:].rearrange("p (g f) -> p g f", f=F),
                    axis=mybir.AxisListType.X)
                k0 += g
                i += 1

        def finale(h, k_lo, k_hi, engines):
            """argmax + gather + decode for keypoints [k_lo, k_hi).
            Writes out rows [k_lo, k_hi) of the output."""
            kh = k_hi - k_lo

            # transpose this half's pmax -> [kh, 128]
            pm = pmax_a if h == 0 else pmax_b
            psumT = psum_pool.tile([kh, P], f32, tag=f"psumT{h}")
            nc.tensor.matmul(psumT[:], pm[:], ident[:],
                             is_transpose=True)
            pT = small_pool.tile([kh, P], f32, tag=f"pT{h}")
            nc.vector.tensor_copy(pT[:], psumT[:])

            gmax = small_pool.tile([kh, 1], f32, tag=f"gmax{h}")
            nc.vector.reduce_max(gmax[:], pT[:], axis=mybir.AxisListType.X)
            cand_p = small_pool.tile([kh, P], f32, tag=f"cand_p{h}")
            nc.vector.scalar_tensor_tensor(
                cand_p[:], in0=pT[:], scalar=gmax[:], in1=riota_p[0:kh, :],
                op0=Alu.is_ge, op1=Alu.mult)
            rp = small_pool.tile([kh, 1], f32, tag=f"rp{h}")  # 128 - p*
            nc.vector.reduce_max(rp[:], cand_p[:], axis=mybir.AxisListType.X)

            # DRAM element offset of winning row k: (128*(k+1) - rp)*2048
            psumR = psum_pool.tile([1, kh], f32, tag=f"psumR{h}")
            nc.tensor.matmul(psumR[:], rp[:], ident[0:kh, 0:kh],
                             is_transpose=True)
            kiota_row = const_pool.tile([1, kh], f32, tag=f"kiota{h}")
            nc.gpsimd.iota(kiota_row[:], pattern=[[P, kh]], base=P * (k_lo + 1),
                           channel_multiplier=0,
                           allow_small_or_imprecise_dtypes=True)
            offs_f = small_pool.tile([1, kh], f32, tag=f"offs_f{h}")
            nc.vector.tensor_sub(offs_f[:], kiota_row[:], psumR[:])
            nc.vector.tensor_scalar(offs_f[:], offs_f[:], float(F), None,
                                    Alu.mult)
            offs_i = small_pool.tile([1, kh], i32, tag=f"offs_i{h}")
            nc.vector.tensor_copy(offs_i[:], offs_f[:])

            grow = small_pool.tile([kh, F], f32, tag=f"grow{h}")
            for k in range(kh):
                eng = engines[k % len(engines)]
                reg = eng.alloc_register()
                eng.load(reg, offs_i[0:1, k:k + 1])
                off = eng.snap(reg, donate=True)
                eng.dma_start(grow[k:k + 1, :], x_flat[bass.ds(off, F)])

            # index within the winning row
            cand_j = small_pool.tile([kh, F], f32, tag=f"cand_j{h}")
            nc.vector.scalar_tensor_tensor(
                cand_j[:], in0=grow[:], scalar=gmax[:], in1=riota_j[0:kh, :],
                op0=Alu.is_ge, op1=Alu.mult)
            rj = small_pool.tile([kh, 1], f32, tag=f"rj{h}")  # 2048 - j*
            nc.vector.reduce_max(rj[:], cand_j[:], axis=mybir.AxisListType.X)

            # decode: flat = p* * 2048 + j* = 264192 - 2048*rp - rj
            flat = small_pool.tile([kh, 1], f32, tag=f"flat{h}")
            nc.vector.scalar_tensor_tensor(
                flat[:], in0=rp[:], scalar=float(F), in1=rj[:],
                op0=Alu.mult, op1=Alu.add)
            nc.vector.tensor_scalar(flat[:], flat[:], -1.0, float(P * F + F),
                                    Alu.mult, Alu.add)
            flat_i = small_pool.tile([kh, 1], i32, tag=f"flat_i{h}")
            nc.vector.tensor_copy(flat_i[:], flat[:])
            xx_i = small_pool.tile([kh, 1], i32, tag=f"xx_i{h}")
            nc.vector.tensor_scalar(xx_i[:], flat_i[:], W - 1, None,
                                    Alu.bitwise_and)
            yy_i = small_pool.tile([kh, 1], i32, tag=f"yy_i{h}")
            nc.vector.tensor_scalar(yy_i[:], flat_i[:], 9, None,
                                    Alu.logical_shift_right)

            conf = small_pool.tile([kh, 1], f32, tag=f"conf{h}")
            nc.scalar.activation(conf[:], gmax[:], Act.Sigmoid)
            valid = small_pool.tile([kh, 1], f32, tag=f"valid{h}")
            nc.vector.tensor_scalar(valid[:], conf[:], 0.8, None, Alu.is_gt)

            # candidate values [4*xx, 4*yy, conf]; defaults (-4,-4,-1)
            cand = small_pool.tile([kh, 3], f32, tag=f"cand{h}")
            nc.vector.tensor_scalar(cand[:, 0:1], xx_i[:], 4.0, None, Alu.mult)
            nc.vector.tensor_scalar(cand[:, 1:2], yy_i[:], 4.0, None, Alu.mult)
            nc.vector.tensor_copy(cand[:, 2:3], conf[:])
            vb3 = small_pool.tile([kh, 3], i32, tag=f"vb3{h}")
            nc.vector.tensor_scalar(vb3[:], cand[:], 0.0, valid[:],
                                    Alu.mult, Alu.add)
            out_sb = small_pool.tile([kh, 3], f32, tag=f"out_sb{h}")
            nc.vector.memset(out_sb[:, 0:2], -4.0)
            nc.vector.memset(out_sb[:, 2:3], -1.0)
            nc.vector.copy_predicated(out_sb[:], vb3[:], cand[:])
            nc.scalar.dma_start(out_dram.ap()[k_lo:k_hi, :], out_sb[:])

        stream(0, KSPLIT)
        # half 1 finale: gather on gpsimd only (SWDGE) so its DMAs don't
        # queue behind half-2 streaming on the sync/scalar HWDGE rings.
        finale(0, 0, KSPLIT, [nc.gpsimd])
        stream(KSPLIT, K)
        finale(1, KSPLIT, K, [nc.sync, nc.scalar])

    nc.compile()
    return nc


def _get_nc():
    global _NC_CACHE
    if _NC_CACHE is None:
        _NC_CACHE = _build()
    return _NC_CACHE


def _shard(x: np.ndarray) -> list[dict[str, np.ndarray]]:
    xf = np.ascontiguousarray(np.asarray(x, dtype=np.float32).reshape(NK, ROW))
    shards = []
    for c in range(N_CORES):
        lo = c * K
        s = xf[lo:min(lo + K, NK)]
        if s.shape[0] < K:
            s = np.concatenate(
                [s, np.zeros((K - s.shape[0], ROW), np.float32)], axis=0)
        shards.append({"x": np.ascontiguousarray(s)})
    return shards


def _run(x, trace=False, **kw):
    nc = _get_nc()
    res = run_bass_kernel_spmd(nc, _shard(x), core_ids=list(range(N_CORES)),
                               trace=trace, **kw)
    out = np.concatenate([r["out"] for r in res.results], axis=0)[:NK]
    return out.astype(np.float32), res


def kernel(x: np.ndarray) -> np.ndarray:
    out, _ = _run(x, trace=False)
    return out
